# revision 2
# baseline (speedup 1.0000x reference)
"""CIN (Compressed Interaction Network) kernel for Trainium2, 8 NeuronCores.

Reference computation (per sample b, NFIELD=64, NEMB=64, NFILTER=128, 3 layers):
    xk_{l+1}[o, e] = relu( sum_{f,c} W_l[o, f*C+c] * x0[f, e] * xk_l[c, e] )
    pooled_l = sum_e xk_{l+1};  y = concat(pooled) @ Wa.T

Strategy (v2):
  - Data-parallel over batch: 32 samples/core, free axis J = 32*64 = 2048
    (b-major, e-minor), pipelined through the layers in 4 free blocks of 512
    (one PSUM bank each).
  - Per layer the GEMM is out = W @ H with H[(f,c), j] = x0[f,j] * xk[c,j].
    H is materialized K-tile by K-tile in bf16 by DVE tensor_tensor; a few
    trailing tiles per layer go to GpSimd (Pool) to offload the DVE.
  - Layer 0 is symmetric (xk = x0): W0 host-folded onto upper-triangle pairs,
    K = 2176 -> 17 K-tiles; operands host-gathered (x0packf/x0packc).
  - Layers 1-2 modulator "hex" tiles (16 fields partition-replicated) are
    built by ONE partition-broadcast DMA each from a compact DRAM row —
    no host-replicated seed array, no SBUF->SBUF doubling chain.
  - xk is written once per layer by ScalarE ReLU; the next layer's TTs repeat
    it 4x along the free axis with a stride-0 access pattern (no xk4 copy).
  - Head: S = sum_l wa_l * xk_l accumulated on DVE (per-partition scalar
    multiply-add), then y = ones^T @ S on the PE + a per-sample reduce.
    This replaces the per-sample ScalarE accum_out pooling entirely.
  - Weights travel on the ScalarE DGE queue so they interleave with the bulk
    (packs/modulators) on the Sync queue instead of queueing behind it.
"""

import sys

if "/opt/trn_rl_repo" not in sys.path:
    sys.path.insert(0, "/opt/trn_rl_repo")

import numpy as np
import ml_dtypes

B, F, E, O = 256, 64, 64, 128
NCORES = 8
BC = B // NCORES          # samples per core
J = BC * E                # free columns per core
JB = 512                  # free-block size (one PSUM bank)
NJ = J // JB              # 4 free blocks
KT0 = 17                  # layer-0 K-tiles (packed symmetric, 2176 = 17*128)
K0 = KT0 * 128
PKW = KT0 * JB            # free width of a packed layer-0 operand tile
HEXW = 16 * JB            # free width of a 16-field modulator tile
POOL_TT = 3               # trailing wide-TTs per (jj, layer>=1) on GpSimd

_BF16 = ml_dtypes.bfloat16
_STATE = {}

_PAIRS = [(f, c) for f in range(F) for c in range(f, F)]
_F_IDX = np.array([p[0] for p in _PAIRS] + [0] * (K0 - len(_PAIRS)), np.int64)
_C_IDX = np.array([p[1] for p in _PAIRS] + [0] * (K0 - len(_PAIRS)), np.int64)


def _build_nc():
    import concourse.bass as bass
    import concourse.tile as tile
    import concourse.mybir as mybir
    from concourse import bacc

    dt = mybir.dt
    Alu = mybir.AluOpType
    nc = bacc.Bacc("TRN2", target_bir_lowering=False, debug=False)

    x0hex = nc.dram_tensor("x0hex", [NJ * 4, HEXW], dt.bfloat16, kind="ExternalInput")
    x0packf = nc.dram_tensor(
        "x0packf", [NJ, 128, PKW], dt.bfloat16, kind="ExternalInput"
    )
    x0packc = nc.dram_tensor(
        "x0packc", [NJ, 128, PKW], dt.bfloat16, kind="ExternalInput"
    )
    w0t = nc.dram_tensor("w0t", [128, KT0 * O], dt.bfloat16, kind="ExternalInput")
    w1t = nc.dram_tensor("w1t", [128, 64 * O], dt.bfloat16, kind="ExternalInput")
    w2t = nc.dram_tensor("w2t", [128, 64 * O], dt.bfloat16, kind="ExternalInput")
    wa = nc.dram_tensor("wa", [O, 3], dt.float32, kind="ExternalInput")
    y = nc.dram_tensor("y", [1, BC], dt.float32, kind="ExternalOutput")

    KT = [KT0, 64, 64]

    with tile.TileContext(nc) as tc:
        with (
            tc.tile_pool(name="wpool", bufs=1) as wpool,
            tc.tile_pool(name="xpool", bufs=1) as xpool,
            tc.tile_pool(name="modpool", bufs=5) as modpool,
            tc.tile_pool(name="packpool", bufs=1) as packpool,
            tc.tile_pool(name="hpool", bufs=6) as hpool,
            tc.tile_pool(name="gpool", bufs=3) as gpool,
            tc.tile_pool(name="xkpool", bufs=3) as xkpool,
            tc.tile_pool(name="spool", bufs=2) as spool,
            tc.tile_pool(name="psum", bufs=3, space="PSUM") as psum_pool,
            tc.tile_pool(name="psumy", bufs=2, space="PSUM") as psumy_pool,
        ):
            w_sb = []
            for li, kt in enumerate(KT):
                w = wpool.tile([128, kt, O], dt.bfloat16, tag=f"w{li}", name=f"w{li}")
                w_sb.append(w)
            # weights on the ScalarE DGE queue: w0 first (layer 0 gate), then
            # wa/w1/w2 trickle in parallel with the Sync-queue bulk
            nc.scalar.dma_start(w_sb[0][:].rearrange("p t o -> p (t o)"), w0t[:])
            wa_sb = xpool.tile([O, 3], dt.float32, tag="wa")
            nc.scalar.dma_start(wa_sb[:], wa[:])
            nc.scalar.dma_start(w_sb[1][:].rearrange("p t o -> p (t o)"), w1t[:])
            nc.scalar.dma_start(w_sb[2][:].rearrange("p t o -> p (t o)"), w2t[:])

            ones = xpool.tile([128, 1], dt.float32, tag="ones")
            nc.vector.memset(ones[:], 1.0)
            ysb = xpool.tile([1, J], dt.float32, tag="ysb")

            for jj in range(NJ):
                # layer-0 packed operand tiles + modulator hex tiles
                p0f = packpool.tile([128, PKW], dt.bfloat16, tag="p0f", name=f"p0f{jj}")
                p0c = packpool.tile([128, PKW], dt.bfloat16, tag="p0c", name=f"p0c{jj}")
                nc.sync.dma_start(p0f[:], x0packf[jj])
                nc.sync.dma_start(p0c[:], x0packc[jj])
                mhs = []
                for hx in range(4):
                    mh = modpool.tile(
                        [128, HEXW], dt.bfloat16, tag="mod", name=f"mh{jj}_{hx}"
                    )
                    nc.sync.dma_start(
                        mh[:], x0hex[4 * jj + hx : 4 * jj + hx + 1].partition_broadcast(128)
                    )
                    mhs.append(mh)

                xk = None
                S = None
                for l in range(3):
                    kt = KT[l]
                    acc = psum_pool.tile(
                        [128, JB], dt.float32, tag="acc", name=f"acc{jj}_{l}"
                    )
                    if l == 0:
                        # 4 wide TTs of 2048 + one of 512 over 17 packed K-tiles
                        for g in range(5):
                            nk = 4 if g < 4 else 1
                            w_ = JB * nk
                            h = hpool.tile(
                                [128, 4 * JB], dt.bfloat16, tag="h", name=f"h0_{jj}_{g}"
                            )
                            nc.vector.tensor_tensor(
                                h[:, 0:w_],
                                p0f[:, 4 * JB * g : 4 * JB * g + w_],
                                p0c[:, 4 * JB * g : 4 * JB * g + w_],
                                op=Alu.mult,
                            )
                            for u in range(nk):
                                t = 4 * g + u
                                nc.tensor.matmul(
                                    acc[:], w_sb[0][:, t, :],
                                    h[:, JB * u : JB * (u + 1)],
                                    start=(t == 0), stop=(t == kt - 1),
                                )
                    else:
                        xk_rep = xk[:].unsqueeze(1).broadcast_to([128, 4, JB])
                        # trailing tiles on GpSimd, issued first so they run
                        # concurrently with the DVE tiles and finish in time
                        gh = []
                        for pp in range(POOL_TT):
                            gi = 16 - POOL_TT + pp
                            hx, s = divmod(gi, 4)
                            hp = gpool.tile(
                                [128, 4 * JB], dt.bfloat16, tag="gh",
                                name=f"gh{jj}_{l}_{pp}",
                            )
                            nc.gpsimd.tensor_tensor(
                                hp[:].rearrange("p (u e) -> p u e", u=4),
                                xk_rep,
                                mhs[hx][:, 4 * JB * s : 4 * JB * (s + 1)].rearrange(
                                    "p (u e) -> p u e", u=4
                                ),
                                op=Alu.mult,
                            )
                            gh.append(hp)
                        for gi in range(16 - POOL_TT):
                            hx, s = divmod(gi, 4)
                            h = hpool.tile(
                                [128, 4 * JB], dt.bfloat16, tag="h",
                                name=f"h{jj}_{l}_{gi}",
                            )
                            nc.vector.tensor_tensor(
                                h[:].rearrange("p (u e) -> p u e", u=4),
                                xk_rep,
                                mhs[hx][:, 4 * JB * s : 4 * JB * (s + 1)].rearrange(
                                    "p (u e) -> p u e", u=4
                                ),
                                op=Alu.mult,
                            )
                            if gi == 4:
                                # fold the previous layer's xk into S off the
                                # critical path (DVE queue, after 5 TTs)
                                if l == 1:
                                    S = spool.tile(
                                        [128, JB], dt.float32, tag="S", name=f"S{jj}"
                                    )
                                    nc.vector.tensor_scalar(
                                        S[:], xk[:], wa_sb[:, 0:1], None, Alu.mult
                                    )
                                else:
                                    nc.vector.scalar_tensor_tensor(
                                        S[:], xk[:], wa_sb[:, 1:2], S[:],
                                        Alu.mult, Alu.add,
                                    )
                            for u in range(4):
                                t = 4 * gi + u
                                nc.tensor.matmul(
                                    acc[:], w_sb[l][:, t, :],
                                    h[:, JB * u : JB * (u + 1)],
                                    start=(t == 0), stop=False,
                                )
                        for pp in range(POOL_TT):
                            for u in range(4):
                                t = 4 * (16 - POOL_TT + pp) + u
                                nc.tensor.matmul(
                                    acc[:], w_sb[l][:, t, :],
                                    gh[pp][:, JB * u : JB * (u + 1)],
                                    start=False, stop=(t == kt - 1),
                                )
                    # epilogue: single ReLU writes xk (unblocks next layer)
                    xk_new = xkpool.tile([128, JB], dt.bfloat16, tag="xk",
                                         name=f"xk{jj}_{l}")
                    nc.scalar.activation(
                        xk_new[:], acc[:], mybir.ActivationFunctionType.Relu
                    )
                    xk = xk_new

                # close S with the last layer's xk, then the head partial:
                # yac[0, j] = sum_o S[o, j]
                nc.vector.scalar_tensor_tensor(
                    S[:], xk[:], wa_sb[:, 2:3], S[:], Alu.mult, Alu.add
                )
                yac = psumy_pool.tile([1, JB], dt.float32, tag="yac", name=f"yac{jj}")
                nc.tensor.matmul(yac[:], ones[:], S[:], start=True, stop=True)
                nc.scalar.copy(ysb[:, JB * jj : JB * (jj + 1)], yac[:])

            # y[b] = sum_e ysb[0, b*E + e]
            y_sb = xpool.tile([1, BC], dt.float32, tag="yout")
            nc.vector.tensor_reduce(
                y_sb[:],
                ysb[:].rearrange("p (b e) -> p b e", b=BC),
                mybir.AxisListType.X,
                Alu.add,
            )
            nc.scalar.dma_start(y[:], y_sb[:])

    nc.finalize()
    return nc


def _get_nc():
    if "nc" not in _STATE:
        _STATE["nc"] = _build_nc()
    return _STATE["nc"]


def _pack_w0(W0):
    # fold symmetric (f, c) weight pairs onto f <= c; pad to K0 with zeros
    w = np.asarray(W0, np.float32).reshape(O, F, F)
    wp = np.zeros((O, K0), np.float32)
    k = 0
    for f in range(F):
        wp[:, k] = w[:, f, f]
        k += 1
        n = F - f - 1
        if n:
            wp[:, k : k + n] = w[:, f, f + 1 :] + w[:, f + 1 :, f]
            k += n
    return wp


def _prep_in_maps(x, W0, W1, W2, Wa):
    x = np.asarray(x, dtype=np.float32)

    def w_layout(wt):
        K = wt.shape[0]
        return np.ascontiguousarray(
            wt.reshape(K // 128, 128, O).transpose(1, 0, 2).reshape(128, -1)
        )

    w0t = w_layout(_pack_w0(W0).T).astype(_BF16)
    w1t = w_layout(np.ascontiguousarray(np.asarray(W1, np.float32).T)).astype(_BF16)
    w2t = w_layout(np.ascontiguousarray(np.asarray(W2, np.float32).T)).astype(_BF16)
    wa = np.ascontiguousarray(np.asarray(Wa, np.float32).reshape(3, O).T)

    def pack_gather(x0b, idx):
        g = x0b[idx]                                        # (K0, J)
        g = g.reshape(KT0, 128, NJ, JB).transpose(2, 1, 0, 3)
        return np.ascontiguousarray(g.reshape(NJ, 128, KT0 * JB))

    in_maps = []
    for c in range(NCORES):
        xc = x[c * BC : (c + 1) * BC]                       # (BC, F, E)
        x0 = np.ascontiguousarray(xc.transpose(1, 0, 2).reshape(F, J))
        x0b = x0.astype(_BF16)
        x0r = x0b.reshape(F, NJ, JB)
        hexrows = np.empty((NJ * 4, HEXW), _BF16)
        for jj in range(NJ):
            for hx in range(4):
                hexrows[4 * jj + hx] = x0r[16 * hx : 16 * hx + 16, jj].reshape(HEXW)
        in_maps.append(
            {
                "x0hex": hexrows,
                "x0packf": pack_gather(x0b, _F_IDX),
                "x0packc": pack_gather(x0b, _C_IDX),
                "w0t": w0t,
                "w1t": w1t,
                "w2t": w2t,
                "wa": wa,
            }
        )
    return in_maps


def _run(inputs, trace=False, **kwargs):
    from concourse.bass_utils import run_bass_kernel_spmd

    nc = _get_nc()
    in_maps = _prep_in_maps(**inputs)
    res = run_bass_kernel_spmd(
        nc, in_maps, core_ids=list(range(NCORES)), trace=trace, **kwargs
    )
    y = np.concatenate(
        [np.asarray(r["y"], np.float32).reshape(BC) for r in res.results]
    )
    return y, res


def kernel(**inputs) -> np.ndarray:
    y, _ = _run(inputs, trace=False)
    return y


# revision 3
# speedup vs baseline: 1.4295x; 1.4295x over previous
"""CIN (Compressed Interaction Network) kernel for Trainium2, 8 NeuronCores.

Reference computation (per sample b, NFIELD=64, NEMB=64, NFILTER=128, 3 layers):
    xk_{l+1}[o, e] = relu( sum_{f,c} W_l[o, f*C+c] * x0[f, e] * xk_l[c, e] )
    pooled_l = sum_e xk_{l+1};  y = concat(pooled) @ Wa.T

Strategy (v3):
  - Data-parallel over batch: 32 samples/core, free axis J = 32*64 = 2048
    (b-major, e-minor), pipelined through the layers in 4 free blocks of 512
    (one PSUM bank each).
  - Per layer the GEMM is out = W @ H with H[(f,c), j] = x0[f,j] * xk[c,j].
    H is materialized K-tile by K-tile in bf16 by DVE tensor_tensor with a
    stride-0 free-dim repeat of the non-materialized operand.
  - Layers 1-2: K-tile t = field t; partition carries c, so xk is read
    directly (stride-0 4x repeat) and only the modulator (x0 rows partition-
    replicated into "hex" tiles of 16 fields) is materialized — one
    partition-broadcast DMA per tile from a compact DRAM row.
  - Layer 0 (symmetric, xk = x0) uses cyclic bands: K-slot (t, p) holds the
    pair (f, c) = (p%64, (p%64 + 2t + p//64) % 64); W0 is host-folded onto
    these slots. The f-side operand is then a fixed [128, JB] tile (x0
    stacked twice, stride-0 repeat) and only the c-side (x0 cyclically
    shifted per tile, host-gathered) is materialized: K = 17 tiles, one
    8.9MB operand instead of two.
  - Head: S = sum_l wa_l * xk_l accumulated on DVE (per-partition scalar
    multiply-add), then y = ones^T @ S on the PE + a per-sample reduce.
  - Weights travel on the ScalarE DGE queue so they interleave with the bulk
    (shift tiles / modulators) on the Sync queue instead of queueing behind.
"""

import sys

if "/opt/trn_rl_repo" not in sys.path:
    sys.path.insert(0, "/opt/trn_rl_repo")

import numpy as np
import ml_dtypes

B, F, E, O = 256, 64, 64, 128
NCORES = 8
BC = B // NCORES          # samples per core
J = BC * E                # free columns per core
JB = 512                  # free-block size (one PSUM bank)
NJ = J // JB              # 4 free blocks
KT0 = 17                  # layer-0 K-tiles (cyclic-band symmetric packing)
K0 = KT0 * 128
PKW = KT0 * JB            # free width of the layer-0 shift operand tile
HEXW = 16 * JB            # free width of a 16-field modulator tile

_BF16 = ml_dtypes.bfloat16
_STATE = {}

# layer-0 cyclic-band layout: slot (t, p) -> (f, c) with d = 2t + p//64
_P = np.arange(128)
_SHIFT_F = _P % 64                                             # (128,)
_SHIFT_D = (2 * np.arange(KT0)[:, None] + _P[None, :] // 64)   # (17, 128)
_SHIFT_C = (_SHIFT_F[None, :] + _SHIFT_D) % 64                 # (17, 128)


def _build_nc():
    import concourse.bass as bass
    import concourse.tile as tile
    import concourse.mybir as mybir
    from concourse import bacc

    dt = mybir.dt
    Alu = mybir.AluOpType
    nc = bacc.Bacc("TRN2", target_bir_lowering=False, debug=False)

    x0hex = nc.dram_tensor("x0hex", [NJ * 4, HEXW], dt.bfloat16, kind="ExternalInput")
    x0shift = nc.dram_tensor(
        "x0shift", [NJ, 128, PKW], dt.bfloat16, kind="ExternalInput"
    )
    x0c = nc.dram_tensor("x0c", [F, J], dt.bfloat16, kind="ExternalInput")
    w0t = nc.dram_tensor("w0t", [128, KT0 * O], dt.bfloat16, kind="ExternalInput")
    w1t = nc.dram_tensor("w1t", [128, 64 * O], dt.bfloat16, kind="ExternalInput")
    w2t = nc.dram_tensor("w2t", [128, 64 * O], dt.bfloat16, kind="ExternalInput")
    wa = nc.dram_tensor("wa", [O, 3], dt.float32, kind="ExternalInput")
    y = nc.dram_tensor("y", [1, BC], dt.float32, kind="ExternalOutput")

    KT = [KT0, 64, 64]

    with tile.TileContext(nc) as tc:
        with (
            tc.tile_pool(name="wpool", bufs=1) as wpool,
            tc.tile_pool(name="xpool", bufs=1) as xpool,
            tc.tile_pool(name="modpool", bufs=6) as modpool,
            tc.tile_pool(name="packpool", bufs=1) as packpool,
            tc.tile_pool(name="stackpool", bufs=2) as stackpool,
            tc.tile_pool(name="hpool", bufs=6) as hpool,
            tc.tile_pool(name="xkpool", bufs=3) as xkpool,
            tc.tile_pool(name="spool", bufs=2) as spool,
            tc.tile_pool(name="psum", bufs=3, space="PSUM") as psum_pool,
            tc.tile_pool(name="psumy", bufs=2, space="PSUM") as psumy_pool,
        ):
            w_sb = []
            for li, kt in enumerate(KT):
                w = wpool.tile([128, kt, O], dt.bfloat16, tag=f"w{li}", name=f"w{li}")
                w_sb.append(w)
            # weights on the ScalarE DGE queue: w0 first (layer 0 gate), then
            # wa/w1/w2 trickle in parallel with the Sync-queue bulk
            nc.scalar.dma_start(w_sb[0][:].rearrange("p t o -> p (t o)"), w0t[:])
            wa_sb = xpool.tile([O, 3], dt.float32, tag="wa")
            nc.scalar.dma_start(wa_sb[:], wa[:])
            nc.scalar.dma_start(w_sb[1][:].rearrange("p t o -> p (t o)"), w1t[:])
            nc.scalar.dma_start(w_sb[2][:].rearrange("p t o -> p (t o)"), w2t[:])

            ones = xpool.tile([128, 1], dt.float32, tag="ones")
            nc.vector.memset(ones[:], 1.0)
            ysb = xpool.tile([1, J], dt.float32, tag="ysb")

            for jj in range(NJ):
                jsl = slice(JB * jj, JB * (jj + 1))
                # layer-0 operands: fixed double-stacked x0 + shifted tiles
                xs2 = stackpool.tile([128, JB], dt.bfloat16, tag="xs2",
                                     name=f"xs2_{jj}")
                nc.sync.dma_start(xs2[0:F, :], x0c[:, jsl])
                nc.sync.dma_start(xs2[F:128, :], x0c[:, jsl])
                p0s = packpool.tile([128, PKW], dt.bfloat16, tag="p0s",
                                    name=f"p0s{jj}")
                nc.sync.dma_start(p0s[:], x0shift[jj])
                mhs = []
                for hx in range(4):
                    mh = modpool.tile(
                        [128, HEXW], dt.bfloat16, tag="mod", name=f"mh{jj}_{hx}"
                    )
                    nc.sync.dma_start(
                        mh[:],
                        x0hex[4 * jj + hx : 4 * jj + hx + 1].partition_broadcast(128),
                    )
                    mhs.append(mh)

                xk = None
                S = None
                for l in range(3):
                    kt = KT[l]
                    acc = psum_pool.tile(
                        [128, JB], dt.float32, tag="acc", name=f"acc{jj}_{l}"
                    )
                    if l == 0:
                        xs2_rep = xs2[:].unsqueeze(1).broadcast_to([128, 4, JB])
                        for g in range(5):
                            nk = 4 if g < 4 else 1
                            h = hpool.tile(
                                [128, 4 * JB], dt.bfloat16, tag="h", name=f"h0_{jj}_{g}"
                            )
                            nc.vector.tensor_tensor(
                                h[:, 0 : JB * nk].rearrange(
                                    "p (u e) -> p u e", u=nk
                                ),
                                xs2[:].unsqueeze(1).broadcast_to([128, nk, JB]),
                                p0s[:, 4 * JB * g : JB * (4 * g + nk)].rearrange(
                                    "p (u e) -> p u e", u=nk
                                ),
                                op=Alu.mult,
                            )
                            for u in range(nk):
                                t = 4 * g + u
                                nc.tensor.matmul(
                                    acc[:], w_sb[0][:, t, :],
                                    h[:, JB * u : JB * (u + 1)],
                                    start=(t == 0), stop=(t == kt - 1),
                                )
                    else:
                        xk_rep = xk[:].unsqueeze(1).broadcast_to([128, 4, JB])
                        for gi in range(16):
                            hx, s = divmod(gi, 4)
                            h = hpool.tile(
                                [128, 4 * JB], dt.bfloat16, tag="h",
                                name=f"h{jj}_{l}_{gi}",
                            )
                            nc.vector.tensor_tensor(
                                h[:].rearrange("p (u e) -> p u e", u=4),
                                xk_rep,
                                mhs[hx][:, 4 * JB * s : 4 * JB * (s + 1)].rearrange(
                                    "p (u e) -> p u e", u=4
                                ),
                                op=Alu.mult,
                            )
                            if gi == 4:
                                # fold the previous layer's xk into S off the
                                # critical path (DVE queue, after 5 TTs)
                                if l == 1:
                                    S = spool.tile(
                                        [128, JB], dt.float32, tag="S", name=f"S{jj}"
                                    )
                                    nc.vector.tensor_scalar(
                                        S[:], xk[:], wa_sb[:, 0:1], None, Alu.mult
                                    )
                                else:
                                    nc.vector.scalar_tensor_tensor(
                                        S[:], xk[:], wa_sb[:, 1:2], S[:],
                                        Alu.mult, Alu.add,
                                    )
                            for u in range(4):
                                t = 4 * gi + u
                                nc.tensor.matmul(
                                    acc[:], w_sb[l][:, t, :],
                                    h[:, JB * u : JB * (u + 1)],
                                    start=(t == 0), stop=(t == kt - 1),
                                )
                    # epilogue: single ReLU writes xk (unblocks next layer)
                    xk_new = xkpool.tile([128, JB], dt.bfloat16, tag="xk",
                                         name=f"xk{jj}_{l}")
                    nc.scalar.activation(
                        xk_new[:], acc[:], mybir.ActivationFunctionType.Relu
                    )
                    xk = xk_new

                # close S with the last layer's xk, then the head partial:
                # yac[0, j] = sum_o S[o, j]
                nc.vector.scalar_tensor_tensor(
                    S[:], xk[:], wa_sb[:, 2:3], S[:], Alu.mult, Alu.add
                )
                yac = psumy_pool.tile([1, JB], dt.float32, tag="yac", name=f"yac{jj}")
                nc.tensor.matmul(yac[:], ones[:], S[:], start=True, stop=True)
                nc.scalar.copy(ysb[:, JB * jj : JB * (jj + 1)], yac[:])

            # y[b] = sum_e ysb[0, b*E + e]
            y_sb = xpool.tile([1, BC], dt.float32, tag="yout")
            nc.vector.tensor_reduce(
                y_sb[:],
                ysb[:].rearrange("p (b e) -> p b e", b=BC),
                mybir.AxisListType.X,
                Alu.add,
            )
            nc.scalar.dma_start(y[:], y_sb[:])

    nc.finalize()
    return nc


def _get_nc():
    if "nc" not in _STATE:
        _STATE["nc"] = _build_nc()
    return _STATE["nc"]


def _pack_w0_cyclic(W0):
    # fold symmetric weights onto the cyclic-band slots (t, p):
    #   d = 2t + p//64, f = p%64, c = (f + d) % 64
    #   d == 0: diagonal; d in 1..31: folded pair; d == 32: f < 32 only;
    #   d >= 33: zero padding
    w = np.asarray(W0, np.float32).reshape(O, F, F)
    wfold = w + w.transpose(0, 2, 1)                 # [O, f, c] folded
    wp = np.zeros((O, KT0, 128), np.float32)
    for t in range(KT0):
        for half in range(2):
            d = 2 * t + half
            if d > 32:
                continue
            p = np.arange(64) + 64 * half
            f = p % 64
            c = (f + d) % 64
            if d == 0:
                wp[:, t, p] = w[:, f, f]
            elif d == 32:
                m = f < 32
                wp[:, t, p[m]] = wfold[:, f[m], c[m]]
            else:
                wp[:, t, p] = wfold[:, f, c]
    return wp.reshape(O, K0)


def _prep_in_maps(x, W0, W1, W2, Wa):
    x = np.asarray(x, dtype=np.float32)

    def w_layout(wt):
        K = wt.shape[0]
        return np.ascontiguousarray(
            wt.reshape(K // 128, 128, O).transpose(1, 0, 2).reshape(128, -1)
        )

    w0t = w_layout(_pack_w0_cyclic(W0).T).astype(_BF16)
    w1t = w_layout(np.ascontiguousarray(np.asarray(W1, np.float32).T)).astype(_BF16)
    w2t = w_layout(np.ascontiguousarray(np.asarray(W2, np.float32).T)).astype(_BF16)
    wa = np.ascontiguousarray(np.asarray(Wa, np.float32).reshape(3, O).T)

    in_maps = []
    for ci in range(NCORES):
        xc = x[ci * BC : (ci + 1) * BC]                     # (BC, F, E)
        x0 = np.ascontiguousarray(xc.transpose(1, 0, 2).reshape(F, J))
        x0b = x0.astype(_BF16)
        x0r = x0b.reshape(F, NJ, JB)
        hexrows = np.empty((NJ * 4, HEXW), _BF16)
        for jj in range(NJ):
            for hx in range(4):
                hexrows[4 * jj + hx] = x0r[16 * hx : 16 * hx + 16, jj].reshape(HEXW)
        # x0shift[jj][p, t*JB + e] = x0[(p%64 + 2t + p//64) % 64, jj*JB + e]
        g = x0r[_SHIFT_C, :, :]                             # (17, 128, NJ, JB)
        shift = np.ascontiguousarray(
            g.transpose(2, 1, 0, 3).reshape(NJ, 128, PKW)
        )
        in_maps.append(
            {
                "x0hex": hexrows,
                "x0shift": shift,
                "x0c": x0b,
                "w0t": w0t,
                "w1t": w1t,
                "w2t": w2t,
                "wa": wa,
            }
        )
    return in_maps


def _run(inputs, trace=False, **kwargs):
    from concourse.bass_utils import run_bass_kernel_spmd

    nc = _get_nc()
    in_maps = _prep_in_maps(**inputs)
    res = run_bass_kernel_spmd(
        nc, in_maps, core_ids=list(range(NCORES)), trace=trace, **kwargs
    )
    y = np.concatenate(
        [np.asarray(r["y"], np.float32).reshape(BC) for r in res.results]
    )
    return y, res


def kernel(**inputs) -> np.ndarray:
    y, _ = _run(inputs, trace=False)
    return y


# revision 5
# speedup vs baseline: 1.5653x; 1.0950x over previous
"""CIN (Compressed Interaction Network) kernel for Trainium2, 8 NeuronCores.

Reference computation (per sample b, NFIELD=64, NEMB=64, NFILTER=128, 3 layers):
    xk_{l+1}[o, e] = relu( sum_{f,c} W_l[o, f*C+c] * x0[f, e] * xk_l[c, e] )
    pooled_l = sum_e xk_{l+1};  y = concat(pooled) @ Wa.T

Strategy (v4):
  - Data-parallel over batch: 32 samples/core, free axis J = 32*64 = 2048
    (b-major, e-minor), in 4 free blocks of 512 (one PSUM bank each).
  - Per layer the GEMM is out = W @ H with H[(f,c), j] = x0[f,j] * xk[c,j],
    materialized in bf16 by DVE tensor_tensor (4096-wide, 8 K-tiles per op)
    with a stride-0 free-dim repeat of the non-materialized operand.
  - Layers 1-2: K-tile t = field t; partition carries c, so xk is read
    directly and only the modulator ("hex" tiles, 16 fields partition-
    replicated) is materialized — one partition-broadcast DMA per tile.
  - Layer 0 (symmetric) uses cyclic bands: slot (t, p) holds the pair
    (f, c) = (p%64, (p%64 + 2t + p//64) % 64); W0 host-folded onto these
    slots. The f-side is a fixed [128, JB] tile (x0 stacked twice, stride-0
    repeat); only the c-side (x0 cyclically shifted, host-gathered) is
    materialized.
  - (jj, layer) blocks are software-pipelined (jj+1's layer 0 sits between
    jj's layer 1 and layer 2) so the DVE never waits out a ReLU->TT layer
    boundary: every boundary is covered by an independent block.
  - Head: ScalarE writes term_l = wa_l * xk_l (per-partition scale); PE
    accumulates y_j = sum_l ones^T @ term_l; final per-sample reduce on DVE.
    No per-sample ScalarE pooling, no DVE involvement in the head.
  - Weights travel on the ScalarE DGE queue so they interleave with the
    Sync-queue bulk instead of queueing behind it.
"""

import sys

if "/opt/trn_rl_repo" not in sys.path:
    sys.path.insert(0, "/opt/trn_rl_repo")

import numpy as np
import ml_dtypes

B, F, E, O = 256, 64, 64, 128
NCORES = 8
BC = B // NCORES          # samples per core
J = BC * E                # free columns per core
JB = 512                  # free-block size (one PSUM bank)
NJ = J // JB              # 4 free blocks
KT0 = 17                  # layer-0 K-tiles (cyclic-band symmetric packing)
K0 = KT0 * 128
PKW = KT0 * JB            # free width of the layer-0 shift operand tile
HEXW = 16 * JB            # free width of a 16-field modulator tile

_BF16 = ml_dtypes.bfloat16
_STATE = {}

# layer-0 cyclic-band layout: slot (t, p) -> (f, c) with d = 2t + p//64
_P = np.arange(128)
_SHIFT_F = _P % 64                                             # (128,)
_SHIFT_D = (2 * np.arange(KT0)[:, None] + _P[None, :] // 64)   # (17, 128)
_SHIFT_C = (_SHIFT_F[None, :] + _SHIFT_D) % 64                 # (17, 128)

# software-pipelined (jj, layer) block order
_BLOCKS = [
    (0, 0), (0, 1), (1, 0), (0, 2), (1, 1), (2, 0),
    (1, 2), (2, 1), (3, 0), (2, 2), (3, 1), (3, 2),
]


def _build_nc():
    import concourse.bass as bass
    import concourse.tile as tile
    import concourse.mybir as mybir
    from concourse import bacc

    dt = mybir.dt
    Alu = mybir.AluOpType
    Act = mybir.ActivationFunctionType
    nc = bacc.Bacc("TRN2", target_bir_lowering=False, debug=False)

    x0hex = nc.dram_tensor("x0hex", [NJ * 4, HEXW], dt.bfloat16, kind="ExternalInput")
    x0shift = nc.dram_tensor(
        "x0shift", [NJ, 128, PKW], dt.bfloat16, kind="ExternalInput"
    )
    x0c = nc.dram_tensor("x0c", [F, J], dt.bfloat16, kind="ExternalInput")
    w0t = nc.dram_tensor("w0t", [128, KT0 * O], dt.bfloat16, kind="ExternalInput")
    w1t = nc.dram_tensor("w1t", [128, 64 * O], dt.bfloat16, kind="ExternalInput")
    w2t = nc.dram_tensor("w2t", [128, 64 * O], dt.bfloat16, kind="ExternalInput")
    wa = nc.dram_tensor("wa", [O, 3], dt.float32, kind="ExternalInput")
    y = nc.dram_tensor("y", [1, BC], dt.float32, kind="ExternalOutput")

    KT = [KT0, 64, 64]
    HW = 8 * JB               # wide-TT width: 8 K-tiles per op

    with tile.TileContext(nc) as tc:
        with (
            tc.tile_pool(name="wpool", bufs=1) as wpool,
            tc.tile_pool(name="xpool", bufs=1) as xpool,
            tc.tile_pool(name="modpool", bufs=6) as modpool,
            tc.tile_pool(name="packpool", bufs=1) as packpool,
            tc.tile_pool(name="stackpool", bufs=2) as stackpool,
            tc.tile_pool(name="hpool", bufs=4) as hpool,
            tc.tile_pool(name="xkpool", bufs=3) as xkpool,
            tc.tile_pool(name="termpool", bufs=4) as termpool,
            tc.tile_pool(name="psum", bufs=4, space="PSUM") as psum_pool,
            tc.tile_pool(name="psumy", bufs=2, space="PSUM") as psumy_pool,
        ):
            w_sb = []
            for li, kt in enumerate(KT):
                w = wpool.tile([128, kt, O], dt.bfloat16, tag=f"w{li}", name=f"w{li}")
                w_sb.append(w)
            # weights on the ScalarE DGE queue: w0 first (layer 0 gate), then
            # wa/w1/w2 trickle in parallel with the Sync-queue bulk
            nc.scalar.dma_start(w_sb[0][:].rearrange("p t o -> p (t o)"), w0t[:])
            wa_sb = xpool.tile([O, 3], dt.float32, tag="wa")
            nc.scalar.dma_start(wa_sb[:], wa[:])
            nc.scalar.dma_start(w_sb[1][:].rearrange("p t o -> p (t o)"), w1t[:])
            nc.scalar.dma_start(w_sb[2][:].rearrange("p t o -> p (t o)"), w2t[:])

            ones = xpool.tile([128, 1], dt.bfloat16, tag="ones")
            nc.vector.memset(ones[:], 1.0)
            ysb = xpool.tile([1, J], dt.float32, tag="ysb")

            # per-jj rolling state
            xs2 = [None] * NJ
            p0s = [None] * NJ
            mhs = [None] * NJ
            xk = [None] * NJ
            terms = [[None] * 3 for _ in range(NJ)]

            def wide_tt(h, w_, in0, in1_tile, in1_off):
                nk = w_ // JB
                nc.vector.tensor_tensor(
                    h[:, 0:w_].rearrange("p (u e) -> p u e", u=nk),
                    in0[:].unsqueeze(1).broadcast_to([128, nk, JB]),
                    in1_tile[:, in1_off : in1_off + w_].rearrange(
                        "p (u e) -> p u e", u=nk
                    ),
                    op=Alu.mult,
                )

            for jj, l in _BLOCKS:
                kt = KT[l]
                if l == 0:
                    # issue this jj's bulk DMAs (first use is here or at l=1)
                    xs2[jj] = stackpool.tile([128, JB], dt.bfloat16, tag="xs2",
                                             name=f"xs2_{jj}")
                    jsl = slice(JB * jj, JB * (jj + 1))
                    nc.sync.dma_start(xs2[jj][0:F, :], x0c[:, jsl])
                    nc.sync.dma_start(xs2[jj][F:128, :], x0c[:, jsl])
                    p0s[jj] = packpool.tile([128, PKW], dt.bfloat16, tag="p0s",
                                            name=f"p0s{jj}")
                    nc.sync.dma_start(p0s[jj][:], x0shift[jj])
                    mhs[jj] = []
                    for hx in range(4):
                        mh = modpool.tile(
                            [128, HEXW], dt.bfloat16, tag="mod", name=f"mh{jj}_{hx}"
                        )
                        nc.sync.dma_start(
                            mh[:],
                            x0hex[
                                4 * jj + hx : 4 * jj + hx + 1
                            ].partition_broadcast(128),
                        )
                        mhs[jj].append(mh)

                acc = psum_pool.tile([128, JB], dt.float32, tag="acc",
                                     name=f"acc{jj}_{l}")
                if l == 0:
                    for g, nk in ((0, 8), (1, 8), (2, 1)):
                        w_ = JB * nk
                        h = hpool.tile([128, HW], dt.bfloat16, tag="h",
                                       name=f"h0_{jj}_{g}")
                        wide_tt(h, w_, xs2[jj], p0s[jj], HW * g)
                        for u in range(nk):
                            t = 8 * g + u
                            nc.tensor.matmul(
                                acc[:], w_sb[0][:, t, :],
                                h[:, JB * u : JB * (u + 1)],
                                start=(t == 0), stop=(t == kt - 1),
                            )
                else:
                    for gi in range(8):
                        h = hpool.tile([128, HW], dt.bfloat16, tag="h",
                                       name=f"h{jj}_{l}_{gi}")
                        wide_tt(h, HW, xk[jj], mhs[jj][gi // 2], HW * (gi % 2))
                        for u in range(8):
                            t = 8 * gi + u
                            nc.tensor.matmul(
                                acc[:], w_sb[l][:, t, :],
                                h[:, JB * u : JB * (u + 1)],
                                start=(t == 0), stop=(t == kt - 1),
                            )
                # epilogue: ReLU writes xk; ScalarE also writes the head term
                xk_new = xkpool.tile([128, JB], dt.bfloat16, tag="xk",
                                     name=f"xk{jj}_{l}")
                nc.scalar.activation(xk_new[:], acc[:], Act.Relu)
                xk[jj] = xk_new
                term = termpool.tile([128, JB], dt.bfloat16, tag="term",
                                     name=f"term{jj}_{l}")
                nc.scalar.activation(
                    term[:], xk_new[:], Act.Copy, scale=wa_sb[:, l : l + 1]
                )
                terms[jj][l] = term

                if l == 2:
                    # head partial: y_j = sum_l ones^T @ term_l
                    yac = psumy_pool.tile([1, JB], dt.float32, tag="yac",
                                          name=f"yac{jj}")
                    for li in range(3):
                        nc.tensor.matmul(
                            yac[:], ones[:], terms[jj][li][:],
                            start=(li == 0), stop=(li == 2),
                        )
                    nc.scalar.copy(ysb[:, JB * jj : JB * (jj + 1)], yac[:])

            # y[b] = sum_e ysb[0, b*E + e]
            y_sb = xpool.tile([1, BC], dt.float32, tag="yout")
            nc.vector.tensor_reduce(
                y_sb[:],
                ysb[:].rearrange("p (b e) -> p b e", b=BC),
                mybir.AxisListType.X,
                Alu.add,
            )
            nc.scalar.dma_start(y[:], y_sb[:])

    nc.finalize()
    return nc


def _get_nc():
    if "nc" not in _STATE:
        _STATE["nc"] = _build_nc()
    return _STATE["nc"]


def _pack_w0_cyclic(W0):
    # fold symmetric weights onto the cyclic-band slots (t, p):
    #   d = 2t + p//64, f = p%64, c = (f + d) % 64
    #   d == 0: diagonal; d in 1..31: folded pair; d == 32: f < 32 only;
    #   d >= 33: zero padding
    w = np.asarray(W0, np.float32).reshape(O, F, F)
    wfold = w + w.transpose(0, 2, 1)                 # [O, f, c] folded
    wp = np.zeros((O, KT0, 128), np.float32)
    for t in range(KT0):
        for half in range(2):
            d = 2 * t + half
            if d > 32:
                continue
            p = np.arange(64) + 64 * half
            f = p % 64
            c = (f + d) % 64
            if d == 0:
                wp[:, t, p] = w[:, f, f]
            elif d == 32:
                m = f < 32
                wp[:, t, p[m]] = wfold[:, f[m], c[m]]
            else:
                wp[:, t, p] = wfold[:, f, c]
    return wp.reshape(O, K0)


def _prep_in_maps(x, W0, W1, W2, Wa):
    x = np.asarray(x, dtype=np.float32)

    def w_layout(wt):
        K = wt.shape[0]
        return np.ascontiguousarray(
            wt.reshape(K // 128, 128, O).transpose(1, 0, 2).reshape(128, -1)
        )

    w0t = w_layout(_pack_w0_cyclic(W0).T).astype(_BF16)
    w1t = w_layout(np.ascontiguousarray(np.asarray(W1, np.float32).T)).astype(_BF16)
    w2t = w_layout(np.ascontiguousarray(np.asarray(W2, np.float32).T)).astype(_BF16)
    wa = np.ascontiguousarray(np.asarray(Wa, np.float32).reshape(3, O).T)

    in_maps = []
    for ci in range(NCORES):
        xc = x[ci * BC : (ci + 1) * BC]                     # (BC, F, E)
        x0 = np.ascontiguousarray(xc.transpose(1, 0, 2).reshape(F, J))
        x0b = x0.astype(_BF16)
        x0r = x0b.reshape(F, NJ, JB)
        hexrows = np.empty((NJ * 4, HEXW), _BF16)
        for jj in range(NJ):
            for hx in range(4):
                hexrows[4 * jj + hx] = x0r[16 * hx : 16 * hx + 16, jj].reshape(HEXW)
        # x0shift[jj][p, t*JB + e] = x0[(p%64 + 2t + p//64) % 64, jj*JB + e]
        g = x0r[_SHIFT_C, :, :]                             # (17, 128, NJ, JB)
        shift = np.ascontiguousarray(
            g.transpose(2, 1, 0, 3).reshape(NJ, 128, PKW)
        )
        in_maps.append(
            {
                "x0hex": hexrows,
                "x0shift": shift,
                "x0c": x0b,
                "w0t": w0t,
                "w1t": w1t,
                "w2t": w2t,
                "wa": wa,
            }
        )
    return in_maps


def _run(inputs, trace=False, **kwargs):
    from concourse.bass_utils import run_bass_kernel_spmd

    nc = _get_nc()
    in_maps = _prep_in_maps(**inputs)
    res = run_bass_kernel_spmd(
        nc, in_maps, core_ids=list(range(NCORES)), trace=trace, **kwargs
    )
    y = np.concatenate(
        [np.asarray(r["y"], np.float32).reshape(BC) for r in res.results]
    )
    return y, res


def kernel(**inputs) -> np.ndarray:
    y, _ = _run(inputs, trace=False)
    return y
